# revision 1
# baseline (speedup 1.0000x reference)
"""Trainium2 Bass kernel v4 for nn_DynamicReindexingRAG (B=4, N=1024, L=128, D=128, Q=64).

Key ideas vs baseline (2.1 ms):
- All doc data fp16 and fully SBUF-RESIDENT (zero per-step HBM traffic):
  dT [d, m] transposed copy for JPE=32 tiles/batch scored on the PE, plus a
  batch-INTERLEAVED normal copy dn4 [q, (tile, batch, d)] for everything else.
- Scores: PE for tiles < JPE (one small matmul per tile x batch), DVE
  (fp16 mult ~0.31 ns/elem + f32 reduce ~1.04 ns/elem) for the rest.
- Pass 2 MERGED across batches: per tile one matmul
  [q,4 w-cols].T @ [q, 4*128 interleaved docs] -> PSUM [4, 512], accumulated
  over all 128 tiles; the useful diagonal blocks give all four O_b. This is
  128 PE matmuls/step instead of 512.
- Cross-core combine identical to the baseline (one AllGather of [1, 520]).
"""

import numpy as np

B, N, L, D, Q = 4, 1024, 128, 128, 64
NL = N * L
N_CORES = 8
MC = NL // N_CORES            # m rows per core per batch (16384)
NTILE = MC // 128             # 128 tiles per batch
JPE = 32                      # tiles per batch scored on the PE
GT = 4                        # tiles per DVE work group

_cache = {}


def build(max_steps: int, n_cores: int = N_CORES):
    import concourse.bass as bass
    import concourse.bacc as bacc
    import concourse.tile as tile
    import concourse.mybir as mybir
    from contextlib import ExitStack

    F32 = mybir.dt.float32
    F16 = mybir.dt.float16
    AF = mybir.ActivationFunctionType
    ALU = mybir.AluOpType
    AX = mybir.AxisListType

    pay = 8 + B * D
    NHI = NTILE - JPE

    nc = bacc.Bacc("TRN2", target_bir_lowering=False, debug=False,
                   num_devices=n_cores)
    dt16_ap = nc.dram_tensor("dt16", [B, D, JPE * 128], F16,
                             kind="ExternalInput").ap()
    dn4_ap = nc.dram_tensor("dn4", [128, NTILE * B * D], F16,
                            kind="ExternalInput").ap()
    a2t_ap = nc.dram_tensor("a2t", [D, D], F32, kind="ExternalInput").ap()
    b2x_ap = nc.dram_tensor("b2x", [D, B], F32, kind="ExternalInput").ap()
    qv0_ap = nc.dram_tensor("qv0", [D, B], F32, kind="ExternalInput").ap()
    ident_ap = nc.dram_tensor("ident", [128, 128], F32,
                              kind="ExternalInput").ap()
    outs_ap = nc.dram_tensor("outs", [max_steps * B, D], F32,
                             kind="ExternalOutput").ap()

    with tile.TileContext(nc) as tc, ExitStack() as ctx:
        const = ctx.enter_context(tc.tile_pool(name="const", bufs=1))
        state = ctx.enter_context(tc.tile_pool(name="state", bufs=1))
        work = ctx.enter_context(tc.tile_pool(name="work", bufs=3))
        small = ctx.enter_context(tc.tile_pool(name="small", bufs=6))
        # PSUM banks: 1 (s) + 1 (o4) + 2 (t) + 2 (m) + 1 (oc) = 7 of 8
        ps_s = ctx.enter_context(tc.tile_pool(name="ps_s", bufs=1, space="PSUM"))
        ps_o = ctx.enter_context(tc.tile_pool(name="ps_o", bufs=1, space="PSUM"))
        ps_t = ctx.enter_context(tc.tile_pool(name="ps_t", bufs=2, space="PSUM"))
        ps_m = ctx.enter_context(tc.tile_pool(name="ps_m", bufs=2, space="PSUM"))
        ps_c = ctx.enter_context(tc.tile_pool(name="ps_c", bufs=1, space="PSUM"))
        dram = ctx.enter_context(tc.tile_pool(name="dram", bufs=1, space="DRAM"))

        # ---- resident docs ----
        dt_sb0 = const.tile([128, JPE * 128], F16)
        nc.sync.dma_start(dt_sb0[:], dt16_ap[0])
        dt_sb1 = const.tile([128, JPE * 128], F16)
        nc.sync.dma_start(dt_sb1[:], dt16_ap[1])
        dt_sb2 = const.tile([128, JPE * 128], F16)
        nc.sync.dma_start(dt_sb2[:], dt16_ap[2])
        dt_sb3 = const.tile([128, JPE * 128], F16)
        nc.sync.dma_start(dt_sb3[:], dt16_ap[3])
        dts = [dt_sb0, dt_sb1, dt_sb2, dt_sb3]
        dn4 = const.tile([128, NTILE * B * D], F16)
        nc.sync.dma_start(dn4[:], dn4_ap[:])

        a2t = const.tile([D, D], F32)
        nc.sync.dma_start(a2t[:], a2t_ap[:])
        b2x = const.tile([D, B], F32)
        nc.sync.dma_start(b2x[:], b2x_ap[:])
        ident = const.tile([128, 128], F32)
        nc.sync.dma_start(ident[:], ident_ap[:])
        ones_row = const.tile([1, 128], F32)
        nc.vector.memset(ones_row[:], 1.0)
        ones_col = const.tile([128, 1], F32)
        nc.vector.memset(ones_col[:], 1.0)

        qv = state.tile([D, B], F32)
        nc.sync.dma_start(qv[:], qv0_ap[:])
        results = state.tile([D, max_steps * B], F32)
        wsum = state.tile([128, 2 * B], F32)

        NR = 2 * n_cores
        cc_in = dram.tile([2, pay], F32)
        cc_out = dram.tile([NR, pay], F32)

        def make_c(qv_src):
            # c = A2 @ qv + b2 -> fp16 cols + interleaved row-broadcast
            cq_ps = ps_m.tile([D, B], F32, tag="m")
            nc.tensor.matmul(cq_ps[:], a2t[:], qv_src[:], start=True, stop=True)
            cf32 = work.tile([D, B], F32, tag="cf32")
            nc.vector.tensor_tensor(cf32[:], cq_ps[:], b2x[:], op=ALU.add)
            cf16 = work.tile([D, B], F16, tag="cf16")
            nc.scalar.copy(cf16[:], cf32[:])
            cb4 = work.tile([128, B * D], F16, tag="cb4")
            for b in range(B):
                ct_ps = ps_t.tile([1, D], F32, tag="t")
                nc.tensor.transpose(ct_ps[:], cf32[:, b:b + 1], ident[:])
                ct_sb = small.tile([1, D], F32, tag="ctsb")
                nc.scalar.copy(ct_sb[:], ct_ps[:])
                cb_ps = ps_t.tile([128, D], F32, tag="t")
                nc.tensor.matmul(cb_ps[:], ones_row[:], ct_sb[:],
                                 start=True, stop=True)
                nc.scalar.copy(cb4[:, b * D:(b + 1) * D], cb_ps[:])
            return cf16, cb4

        cf16, cb4 = make_c(qv)

        for t in range(max_steps):
            # ---- DVE pass 1b, first half of groups (overlaps PE 1a) ----
            ngrp = NHI // GT
            s_dv = work.tile([128, NHI * B], F32, tag="sdv", bufs=2)

            def dve_group(g):
                src = dn4[:, (JPE + g * GT) * B * D:(JPE + (g + 1) * GT) * B * D]
                prod = work.tile([128, GT * B * D], F16, tag="prod", bufs=2)
                ch3 = src.rearrange("p (j bd) -> p j bd", j=GT)
                pr3 = prod[:].rearrange("p (j bd) -> p j bd", j=GT)
                cb3 = cb4[:].rearrange("p (o bd) -> p o bd", o=1)
                i0, i1 = bass.broadcast_tensor_aps(ch3, cb3)
                nc.vector.tensor_tensor(pr3, i0, i1, op=ALU.mult)
                nc.vector.tensor_reduce(
                    s_dv[:, g * GT * B:(g + 1) * GT * B],
                    prod[:].rearrange("p (jb d) -> p jb d", d=D),
                    axis=AX.X, op=ALU.add)

            for g in range(ngrp // 2):
                dve_group(g)

            # ---- pass 1a: PE scores, tiles < JPE; s_pe[q, (b, j)] ----
            s_pe = ps_s.tile([128, B * JPE], F32, tag="s")
            for b in range(B):
                dtb = dts[b]
                for j in range(JPE):
                    nc.tensor.matmul(
                        s_pe[:, b * JPE + j:b * JPE + j + 1],
                        dtb[:, j * 128:(j + 1) * 128],
                        cf16[:, b:b + 1],
                        start=True, stop=True)

            def half_softmax(tag, mpart_view, exp_fn, wsum_cols):
                # local max -> [B,1] col, [1,B] row, neg broadcast; exp; S
                mp = small.tile([128, B], F32, tag="mp" + tag)
                nc.vector.tensor_reduce(mp[:], mpart_view, axis=AX.X,
                                        op=ALU.max)
                mt_ps = ps_t.tile([B, 128], F32, tag="t")
                nc.tensor.transpose(mt_ps[:], mp[:], ident[:])
                M = small.tile([B, 1], F32, tag="M" + tag)
                nc.vector.tensor_reduce(M[:], mt_ps[:], axis=AX.X, op=ALU.max)
                Mrow_ps = ps_t.tile([1, B], F32, tag="t")
                nc.tensor.transpose(Mrow_ps[:], M[:], ident[0:B, 0:B])
                Mrow = small.tile([1, B], F32, tag="Mr" + tag)
                nc.scalar.copy(Mrow[:], Mrow_ps[:])
                negMrow = small.tile([1, B], F32, tag="nMr" + tag)
                nc.scalar.mul(negMrow[:], Mrow_ps[:], -1.0)
                nM_ps = ps_m.tile([128, B], F32, tag="m")
                nc.tensor.matmul(nM_ps[:], ones_row[:], negMrow[:],
                                 start=True, stop=True)
                negMbc = small.tile([128, B], F32, tag="nMbc" + tag)
                nc.scalar.copy(negMbc[:], nM_ps[:])
                exp_fn(negMbc)
                S_ps = ps_m.tile([B, 1], F32, tag="m")
                nc.tensor.matmul(S_ps[:], wsum[:, wsum_cols[0]:wsum_cols[1]],
                                 ones_col[:], start=True, stop=True)
                S_sb = small.tile([B, 1], F32, tag="S" + tag)
                nc.vector.tensor_copy(S_sb[:], S_ps[:])
                Srow_ps = ps_t.tile([1, B], F32, tag="t")
                nc.tensor.transpose(Srow_ps[:], S_sb[:], ident[0:B, 0:B])
                return Mrow, Srow_ps

            def make_payload(tag, Mrow, Srow_ps, o4_ps):
                payl = work.tile([1, pay], F32, tag="pay" + tag, bufs=2)
                nc.vector.tensor_copy(payl[0:1, 0:B], Mrow[:])
                nc.scalar.copy(payl[0:1, B:2 * B], Srow_ps[:])
                o4_sb = work.tile([B, B * D], F32, tag="o4sb" + tag, bufs=1)
                nc.scalar.copy(o4_sb[:], o4_ps[:])
                for b in range(B):
                    orow_ps = ps_m.tile([1, B * D], F32, tag="m")
                    nc.tensor.matmul(orow_ps[:], ident[0:B, b:b + 1],
                                     o4_sb[:], start=True, stop=True)
                    nc.scalar.copy(payl[0:1, 8 + b * D:8 + (b + 1) * D],
                                   orow_ps[0:1, b * D:(b + 1) * D])
                return payl

            # ---- half A: PE-scored tiles ----
            w_pe = work.tile([128, B * JPE], F16, tag="wpe")

            def exp_a(negMbc):
                for b in range(B):
                    nc.scalar.activation(
                        w_pe[:, b * JPE:(b + 1) * JPE],
                        s_pe[:, b * JPE:(b + 1) * JPE], AF.Exp,
                        bias=negMbc[:, b:b + 1], scale=1.0,
                        accum_out=wsum[:, b:b + 1])

            MrowA, SrowA_ps = half_softmax(
                "A", s_pe[:].rearrange("p (b j) -> p b j", b=B),
                exp_a, (0, B))
            o4a_ps = ps_o.tile([B, B * D], F32, tag="o4a", bufs=1)
            w_pe3 = w_pe[:].rearrange("p (b j) -> p j b", b=B)
            for j in range(JPE):
                nc.tensor.matmul(
                    o4a_ps[:], w_pe3[:, j, :],
                    dn4[:, j * B * D:(j + 1) * B * D],
                    start=(j == 0), stop=(j == JPE - 1))
            pay_a = make_payload("a", MrowA, SrowA_ps, o4a_ps)
            nc.gpsimd.dma_start(cc_in[0:1, :], pay_a[:])

            # ---- DVE pass 1b, second half ----
            for g in range(ngrp // 2, ngrp):
                dve_group(g)

            # ---- half B: DVE-scored tiles ----
            w_dv = work.tile([128, NHI * B], F16, tag="wdv")

            def exp_b(negMbc):
                for b in range(B):
                    sdv_b = s_dv[:].rearrange("p (j b) -> p j b", b=B)[:, :, b]
                    wdv_b = w_dv[:].rearrange("p (j b) -> p j b", b=B)[:, :, b]
                    nc.scalar.activation(
                        wdv_b, sdv_b, AF.Exp,
                        bias=negMbc[:, b:b + 1], scale=1.0,
                        accum_out=wsum[:, B + b:B + b + 1])

            MrowB, SrowB_ps = half_softmax(
                "B", s_dv[:].rearrange("p (j b) -> p b j", b=B),
                exp_b, (B, 2 * B))
            o4b_ps = ps_o.tile([B, B * D], F32, tag="o4b", bufs=1)
            w_dv3 = w_dv[:].rearrange("p (j b) -> p j b", b=B)
            for j in range(JPE, NTILE):
                nc.tensor.matmul(
                    o4b_ps[:], w_dv3[:, j - JPE, :],
                    dn4[:, j * B * D:(j + 1) * B * D],
                    start=(j == JPE), stop=(j == NTILE - 1))
            pay_b = make_payload("b", MrowB, SrowB_ps, o4b_ps)
            nc.gpsimd.dma_start(cc_in[1:2, :], pay_b[:])

            # ---- cross-core combine over NR = 2*n_cores virtual rows ----
            nc.gpsimd.collective_compute(
                "AllGather", mybir.AluOpType.bypass,
                replica_groups=[list(range(n_cores))],
                ins=[cc_in.opt()], outs=[cc_out.opt()])
            gath = work.tile([NR, pay], F32, tag="gath", bufs=2)
            nc.gpsimd.dma_start(gath[:], cc_out[:])

            gmT_ps = ps_t.tile([B, NR], F32, tag="t")
            nc.tensor.transpose(gmT_ps[:], gath[:, 0:B], ident[0:NR, 0:NR])
            gmT = small.tile([B, NR], F32, tag="gmT")
            nc.scalar.copy(gmT[:], gmT_ps[:])
            Mg = small.tile([B, 1], F32, tag="Mg")
            nc.vector.tensor_reduce(Mg[:], gmT[:], axis=AX.X, op=ALU.max)
            MgT_ps = ps_t.tile([1, B], F32, tag="t")
            nc.tensor.transpose(MgT_ps[:], Mg[:], ident[0:B, 0:B])
            negMgT = small.tile([1, B], F32, tag="negMgT")
            nc.scalar.mul(negMgT[:], MgT_ps[:], -1.0)
            negMg_ps = ps_m.tile([NR, B], F32, tag="m")
            nc.tensor.matmul(negMg_ps[:], ones_row[0:1, 0:NR],
                             negMgT[:], start=True, stop=True)
            shift = small.tile([NR, B], F32, tag="shift")
            nc.vector.tensor_tensor(shift[:], gath[:, 0:B], negMg_ps[:],
                                    op=ALU.add)
            f_mat = small.tile([NR, B], F32, tag="f_mat")
            nc.scalar.activation(f_mat[:], shift[:], AF.Exp)

            Sf = small.tile([NR, B], F32, tag="Sf")
            nc.vector.tensor_tensor(Sf[:], gath[:, B:2 * B], f_mat[:],
                                    op=ALU.mult)
            St4_ps = ps_m.tile([B, 1], F32, tag="m")
            nc.tensor.matmul(St4_ps[:], Sf[:], ones_col[0:NR, :],
                             start=True, stop=True)
            rS4 = small.tile([B, 1], F32, tag="rS4")
            nc.vector.reciprocal(rS4[:], St4_ps[:])
            rSrow_ps = ps_t.tile([1, B], F32, tag="t")
            nc.tensor.transpose(rSrow_ps[:], rS4[:], ident[0:B, 0:B])
            rSrow = small.tile([1, B], F32, tag="rSrow")
            nc.scalar.copy(rSrow[:], rSrow_ps[:])
            rS8_ps = ps_m.tile([NR, B], F32, tag="m")
            nc.tensor.matmul(rS8_ps[:], ones_row[0:1, 0:NR], rSrow[:],
                             start=True, stop=True)
            f2 = small.tile([NR, B], F32, tag="f2")
            nc.vector.tensor_tensor(f2[:], f_mat[:], rS8_ps[:], op=ALU.mult)
            oc_ps = ps_c.tile([128, B], F32, tag="occ")
            for b in range(B):
                nc.tensor.matmul(oc_ps[:, b:b + 1],
                                 gath[:, 8 + b * D:8 + (b + 1) * D],
                                 f2[:, b:b + 1], start=True, stop=True)

            nc.vector.tensor_copy(results[:, t * B:(t + 1) * B], oc_ps[:])
            qs = work.tile([D, B], F32, tag="qs")
            nc.vector.tensor_tensor(qs[:], qv[:], oc_ps[:], op=ALU.add)
            nc.vector.tensor_scalar(qv[:], qs[:], 0.5, None, op0=ALU.mult)
            if t + 1 < max_steps:
                cf16, cb4 = make_c(qv)

        n_out = max_steps * B
        res_ps = ps_t.tile([n_out, D], F32, tag="t")
        nc.tensor.transpose(res_ps[:], results[:], ident[:])
        res_T = work.tile([n_out, D], F32, tag="resT")
        nc.scalar.copy(res_T[:], res_ps[:])
        nc.sync.dma_start(outs_ap[:], res_T[:])

    nc.compile()
    return nc


def make_inputs(query, documents, Wq, bq, Wk, bk, n_cores: int = N_CORES):
    query = np.asarray(query)
    documents = np.asarray(documents)
    Wq64 = np.asarray(Wq, dtype=np.float64)
    bq64 = np.asarray(bq, dtype=np.float64)
    Wk64 = np.asarray(Wk, dtype=np.float64)

    A2 = Q * (Wk64.T @ Wq64)
    b2 = Q * (Wk64.T @ bq64)
    a2t = np.ascontiguousarray(A2.T.astype(np.float32))
    b2x = np.ascontiguousarray(
        np.repeat(b2.astype(np.float32)[:, None], B, axis=1))
    qv0 = np.ascontiguousarray(
        query.astype(np.float64).mean(axis=1).T.astype(np.float32))
    ident = np.eye(128, dtype=np.float32)

    nl = documents.shape[1] * documents.shape[2]
    dflat = documents.reshape(B, nl, D)
    in_maps = []
    for c in range(n_cores):
        shard = dflat[:, c * MC:(c + 1) * MC, :]
        dt16 = np.ascontiguousarray(
            shard[:, :JPE * 128, :].transpose(0, 2, 1).astype(np.float16))
        # dn4[q, (tile, batch, d)] = docs[b, tile*128+q, d]
        dn4 = np.ascontiguousarray(
            shard.reshape(B, NTILE, 128, D).transpose(2, 1, 0, 3)
            .reshape(128, NTILE * B * D).astype(np.float16))
        in_maps.append({"dt16": dt16, "dn4": dn4, "a2t": a2t, "b2x": b2x,
                        "qv0": qv0, "ident": ident})
    return in_maps


def kernel(query, documents, Wq, bq, Wk, bk, max_steps):
    import time
    from concourse.bass_utils import run_bass_kernel_spmd

    steps = int(max_steps)
    if steps not in _cache:
        _cache[steps] = build(steps)
    nc = _cache[steps]

    in_maps = make_inputs(query, documents, Wq, bq, Wk, bk)
    last_exc = None
    for attempt in range(3):
        try:
            res = run_bass_kernel_spmd(nc, in_maps,
                                       core_ids=list(range(N_CORES)))
            break
        except Exception as e:  # noqa: BLE001
            last_exc = e
            time.sleep(15)
    else:
        raise last_exc
    outs = res.results[0]["outs"]
    return np.ascontiguousarray(
        outs.reshape(steps, B, D).transpose(1, 0, 2))



# revision 2
# speedup vs baseline: 1.4667x; 1.4667x over previous
"""Trainium2 Bass kernel v6 for nn_DynamicReindexingRAG (B=4, N=1024, L=128, D=128, Q=64).

Changes vs v5 (1.58 ms):
- Pool (gpsimd) XYZWC reduces compute per-batch max M_b and sum S_b in one
  instruction each — removes the transpose-heavy DVE max chains.
- Two collectives per step: half A's AllGather is issued mid-step and hides
  under DVE scoring; only half B's is on the critical tail.
- b-major s_dv/w_dv layouts: contiguous ACT exp reads/writes.
- DVE/ACT reduce split with deeper prod rotation (bufs=3) so ACT reduce
  latency doesn't stall DVE multiplies.
"""

import numpy as np

B, N, L, D, Q = 4, 1024, 128, 128, 64
NL = N * L
N_CORES = 8
MC = NL // N_CORES            # m rows per core per batch (16384)
NTILE = MC // 128             # 128 tiles per batch
JPE = 52                      # tiles per batch scored on the PE
NHI = NTILE - JPE             # tiles per batch scored on DVE

_cache = {}


def build(max_steps: int, n_cores: int = N_CORES, jpe: int = JPE, abl=()):
    abl = set(abl)
    import concourse.bass as bass
    import concourse.bacc as bacc
    import concourse.tile as tile
    import concourse.mybir as mybir
    from contextlib import ExitStack

    F32 = mybir.dt.float32
    F16 = mybir.dt.float16
    AF = mybir.ActivationFunctionType
    ALU = mybir.AluOpType
    AX = mybir.AxisListType

    nhi = NTILE - jpe
    pay = 8 + B * D
    GT = 2                        # tiles per DVE work group
    ngrp = nhi // GT

    nc = bacc.Bacc("TRN2", target_bir_lowering=False, debug=False,
                   num_devices=n_cores)
    dt16_ap = nc.dram_tensor("dt16", [B, D, jpe * 128], F16,
                             kind="ExternalInput").ap()
    dn4_ap = nc.dram_tensor("dn4", [128, NTILE * B * D], F16,
                            kind="ExternalInput").ap()
    a2t_ap = nc.dram_tensor("a2t", [D, D], F32, kind="ExternalInput").ap()
    b2x_ap = nc.dram_tensor("b2x", [D, B], F32, kind="ExternalInput").ap()
    qv0_ap = nc.dram_tensor("qv0", [D, B], F32, kind="ExternalInput").ap()
    ident_ap = nc.dram_tensor("ident", [128, 128], F32,
                              kind="ExternalInput").ap()
    outs_ap = nc.dram_tensor("outs", [max_steps * B, D], F32,
                             kind="ExternalOutput").ap()

    with tile.TileContext(nc) as tc, ExitStack() as ctx:
        const = ctx.enter_context(tc.tile_pool(name="const", bufs=1))
        state = ctx.enter_context(tc.tile_pool(name="state", bufs=1))
        work = ctx.enter_context(tc.tile_pool(name="work", bufs=1))
        small = ctx.enter_context(tc.tile_pool(name="small", bufs=3))
        ps_s = ctx.enter_context(tc.tile_pool(name="ps_s", bufs=1, space="PSUM"))
        ps_o = ctx.enter_context(tc.tile_pool(name="ps_o", bufs=1, space="PSUM"))
        ps_t = ctx.enter_context(tc.tile_pool(name="ps_t", bufs=2, space="PSUM"))
        ps_m = ctx.enter_context(tc.tile_pool(name="ps_m", bufs=2, space="PSUM"))
        ps_c = ctx.enter_context(tc.tile_pool(name="ps_c", bufs=1, space="PSUM"))
        dram = ctx.enter_context(tc.tile_pool(name="dram", bufs=1, space="DRAM"))

        # ---- resident docs ----
        dts = []
        for b in range(B):
            dtb = const.tile([128, jpe * 128], F16, tag=f"dtb{b}")
            nc.sync.dma_start(dtb[:], dt16_ap[b])
            dts.append(dtb)
        dn4 = const.tile([128, NTILE * B * D], F16)
        nc.sync.dma_start(dn4[:], dn4_ap[:])

        a2t = const.tile([D, D], F32)
        nc.sync.dma_start(a2t[:], a2t_ap[:])
        b2x = const.tile([D, B], F32)
        nc.sync.dma_start(b2x[:], b2x_ap[:])
        ident = const.tile([128, 128], F32)
        nc.sync.dma_start(ident[:], ident_ap[:])
        ones_row = const.tile([1, 128], F32)
        nc.vector.memset(ones_row[:], 1.0)
        ones_col = const.tile([128, 1], F32)
        nc.vector.memset(ones_col[:], 1.0)

        qv = state.tile([D, B], F32)
        nc.sync.dma_start(qv[:], qv0_ap[:])
        results = state.tile([D, max_steps * B], F32)
        wsum = state.tile([128, 2 * B], F32)

        NR = 2 * n_cores
        cc_in_a = dram.tile([1, pay], F32, tag="cca_i")
        cc_out_a = dram.tile([n_cores, pay], F32, tag="cca_o")
        cc_in_b = dram.tile([1, pay], F32, tag="ccb_i")
        cc_out_b = dram.tile([n_cores, pay], F32, tag="ccb_o")

        def make_c(qv_src):
            # c = A2 @ qv + b2 -> fp16 cols + interleaved row-broadcast
            cq_ps = ps_m.tile([D, B], F32, tag="m")
            nc.tensor.matmul(cq_ps[:], a2t[:], qv_src[:], start=True, stop=True)
            cf32 = work.tile([D, B], F32, tag="cf32")
            nc.vector.tensor_tensor(cf32[:], cq_ps[:], b2x[:], op=ALU.add)
            cf16 = work.tile([D, B], F16, tag="cf16")
            nc.scalar.copy(cf16[:], cf32[:])
            # cb4[q, (b, d)] = c[d, b] broadcast over partitions q
            cb4 = work.tile([128, B * D], F16, tag="cb4", bufs=2)
            for b in range(B):
                ct_ps = ps_t.tile([1, D], F32, tag="t")
                nc.tensor.transpose(ct_ps[:], cf32[:, b:b + 1], ident[:])
                ct_sb = small.tile([1, D], F32, tag="ctsb")
                nc.scalar.copy(ct_sb[:], ct_ps[:])
                cb_ps = ps_t.tile([128, D], F32, tag="t")
                nc.tensor.matmul(cb_ps[:], ones_row[:], ct_sb[:],
                                 start=True, stop=True)
                nc.scalar.copy(cb4[:, b * D:(b + 1) * D], cb_ps[:])
            return cf16, cb4

        cf16, cb4 = make_c(qv)

        for t in range(max_steps):
            # ---- pass 1b: DVE mults; reduces split DVE/ACT; b-major s_dv ----
            s_dv = work.tile([128, B * nhi], F32, tag="sdv", bufs=1)

            def dve_group(g):
                j0 = jpe + g * GT
                src = dn4[:, j0 * B * D:(j0 + GT) * B * D]
                prod = work.tile([128, GT * B * D], F16, tag="prod", bufs=3)
                ch3 = src.rearrange("p (j bd) -> p j bd", j=GT)
                pr3 = prod[:].rearrange("p (j bd) -> p j bd", j=GT)
                cb3 = cb4[:].rearrange("p (o bd) -> p o bd", o=1)
                i0, i1 = bass.broadcast_tensor_aps(ch3, cb3)
                nc.vector.tensor_tensor(pr3, i0, i1, op=ALU.mult)
                pr4 = prod[:].rearrange("p (j b d) -> p j b d", j=GT, b=B)
                if 'dve' in abl:
                    return
                if g % 3 != 2:
                    # reduce on DVE: per batch, segmented over group tiles
                    for b in range(B):
                        nc.vector.tensor_reduce(
                            s_dv[:, b * nhi + g * GT:b * nhi + (g + 1) * GT],
                            pr4[:, :, b, :], axis=AX.X, op=ALU.add)
                else:
                    # reduce on ACT: copy-with-accumulate per tile-batch
                    for jj in range(GT):
                        for b in range(B):
                            scr = work.tile([128, D], F16, tag="scr", bufs=2)
                            nc.scalar.activation(
                                scr[:], prod[:, (jj * B + b) * D:
                                             (jj * B + b + 1) * D], AF.Copy,
                                accum_out=s_dv[:, b * nhi + g * GT + jj:
                                               b * nhi + g * GT + jj + 1])

            if 'dve' in abl:
                nc.vector.memset(s_dv[:], 0.0)
            for g in range(ngrp // 2):
                dve_group(g)

            # ---- pass 1a: PE scores, tiles < jpe; s_pe[q, (b, j)] ----
            s_pe = ps_s.tile([128, B * jpe], F32, tag="s")
            if 'pe1' not in abl:
                for b in range(B):
                    dtb = dts[b]
                    for j in range(jpe):
                        nc.tensor.matmul(
                            s_pe[:, b * jpe + j:b * jpe + j + 1],
                            dtb[:, j * 128:(j + 1) * 128],
                            cf16[:, b:b + 1],
                            start=True, stop=True)
            else:
                nc.tensor.matmul(s_pe[:, 0:1], dts[0][:, 0:128],
                                 cf16[:, 0:1], start=True, stop=True)

            def half_softmax(tag, per_b_views, exp_fn, wsum_cols):
                # per-b full max via Pool XYZWC; neg broadcast; exp; S
                Mrow = small.tile([1, B], F32, tag="Mr" + tag)
                for b in range(B):
                    nc.gpsimd.tensor_reduce(Mrow[0:1, b:b + 1],
                                            per_b_views[b],
                                            axis=AX.XYZWC, op=ALU.max)
                negMrow = small.tile([1, B], F32, tag="nMr" + tag)
                nc.scalar.mul(negMrow[:], Mrow[:], -1.0)
                nM_ps = ps_m.tile([128, B], F32, tag="m")
                nc.tensor.matmul(nM_ps[:], ones_row[:], negMrow[:],
                                 start=True, stop=True)
                negMbc = small.tile([128, B], F32, tag="nMbc" + tag)
                nc.scalar.copy(negMbc[:], nM_ps[:])
                exp_fn(negMbc)
                S4 = small.tile([1, B], F32, tag="S" + tag)
                for b in range(B):
                    nc.gpsimd.tensor_reduce(
                        S4[0:1, b:b + 1],
                        wsum[:, wsum_cols[0] + b:wsum_cols[0] + b + 1],
                        axis=AX.XYZWC, op=ALU.add)
                return Mrow, S4

            def make_payload(tag, Mrow, S4, o4_ps):
                payl = work.tile([1, pay], F32, tag="pay" + tag, bufs=1)
                nc.vector.tensor_copy(payl[0:1, 0:B], Mrow[:])
                nc.vector.tensor_copy(payl[0:1, B:2 * B], S4[:])
                o4_sb = work.tile([B, B * D], F32, tag="o4sb" + tag, bufs=1)
                nc.scalar.copy(o4_sb[:], o4_ps[:])
                for b in range(B):
                    orow_ps = ps_m.tile([1, B * D], F32, tag="m")
                    nc.tensor.matmul(orow_ps[:], ident[0:B, b:b + 1],
                                     o4_sb[:], start=True, stop=True)
                    nc.scalar.copy(payl[0:1, 8 + b * D:8 + (b + 1) * D],
                                   orow_ps[0:1, b * D:(b + 1) * D])
                return payl

            # ---- half A: PE-scored tiles ----
            w_pe = work.tile([128, B * jpe], F16, tag="wpe", bufs=1)
            s_pe16 = work.tile([128, B * jpe], F16, tag="spe16", bufs=1)
            nc.scalar.copy(s_pe16[:], s_pe[:])

            def exp_a(negMbc):
                if 'exp' in abl:
                    nc.vector.memset(w_pe[:], 0.0)
                    nc.vector.memset(wsum[:, 0:B], 1.0)
                    return
                for b in range(B):
                    nc.scalar.activation(
                        w_pe[:, b * jpe:(b + 1) * jpe],
                        s_pe[:, b * jpe:(b + 1) * jpe], AF.Exp,
                        bias=negMbc[:, b:b + 1], scale=1.0,
                        accum_out=wsum[:, b:b + 1])

            MrowA, S4A = half_softmax(
                "A", [s_pe16[:, b * jpe:(b + 1) * jpe] for b in range(B)],
                exp_a, (0, B))
            o4a_ps = ps_o.tile([B, B * D], F32, tag="o4a", bufs=1)
            w_pe3 = w_pe[:].rearrange("p (b j) -> p j b", b=B)
            o4a_range = [0] if 'o4' in abl else list(range(jpe))
            for j in o4a_range:
                nc.tensor.matmul(
                    o4a_ps[:], w_pe3[:, j, :],
                    dn4[:, j * B * D:(j + 1) * B * D],
                    start=(j == o4a_range[0]), stop=(j == o4a_range[-1]))
            pay_a = make_payload("a", MrowA, S4A, o4a_ps)
            nc.gpsimd.dma_start(cc_in_a[0:1, :], pay_a[:])
            if 'cc' not in abl:
                nc.gpsimd.collective_compute(
                    "AllGather", mybir.AluOpType.bypass,
                    replica_groups=[list(range(n_cores))],
                    ins=[cc_in_a.opt()], outs=[cc_out_a.opt()])

            for g in range(ngrp // 2, ngrp):
                dve_group(g)

            # ---- half B: DVE-scored tiles (b-major layouts) ----
            w_dv = work.tile([128, B * nhi], F16, tag="wdv", bufs=1)

            def exp_b(negMbc):
                if 'exp' in abl:
                    nc.vector.memset(w_dv[:], 0.0)
                    nc.vector.memset(wsum[:, B:2 * B], 1.0)
                    return
                for b in range(B):
                    nc.scalar.activation(
                        w_dv[:, b * nhi:(b + 1) * nhi],
                        s_dv[:, b * nhi:(b + 1) * nhi], AF.Exp,
                        bias=negMbc[:, b:b + 1], scale=1.0,
                        accum_out=wsum[:, B + b:B + b + 1])

            MrowB, S4B = half_softmax(
                "B", [s_dv[:, b * nhi:(b + 1) * nhi] for b in range(B)],
                exp_b, (B, 2 * B))
            o4b_ps = ps_o.tile([B, B * D], F32, tag="o4b", bufs=1)
            w_dv3 = w_dv[:].rearrange("p (b j) -> p j b", b=B)
            o4b_range = [jpe] if 'o4' in abl else list(range(jpe, NTILE))
            for j in o4b_range:
                nc.tensor.matmul(
                    o4b_ps[:], w_dv3[:, j - jpe, :],
                    dn4[:, j * B * D:(j + 1) * B * D],
                    start=(j == o4b_range[0]), stop=(j == o4b_range[-1]))
            pay_b = make_payload("b", MrowB, S4B, o4b_ps)
            nc.gpsimd.dma_start(cc_in_b[0:1, :], pay_b[:])
            if 'cc' not in abl:
                nc.gpsimd.collective_compute(
                    "AllGather", mybir.AluOpType.bypass,
                    replica_groups=[list(range(n_cores))],
                    ins=[cc_in_b.opt()], outs=[cc_out_b.opt()])

            # ---- cross-core combine over NR = 2*n_cores virtual rows ----
            gath = work.tile([NR, pay], F32, tag="gath", bufs=1)
            nc.gpsimd.dma_start(gath[0:n_cores, :], cc_out_a[:])
            nc.gpsimd.dma_start(gath[n_cores:NR, :], cc_out_b[:])

            Mgrow = small.tile([1, B], F32, tag="Mgrow")
            for b in range(B):
                nc.gpsimd.tensor_reduce(Mgrow[0:1, b:b + 1], gath[:, b:b + 1],
                                        axis=AX.XYZWC, op=ALU.max)
            negMgT = small.tile([1, B], F32, tag="negMgT")
            nc.scalar.mul(negMgT[:], Mgrow[:], -1.0)
            negMg_ps = ps_m.tile([NR, B], F32, tag="m")
            nc.tensor.matmul(negMg_ps[:], ones_row[0:1, 0:NR],
                             negMgT[:], start=True, stop=True)
            shift = small.tile([NR, B], F32, tag="shift")
            nc.vector.tensor_tensor(shift[:], gath[:, 0:B], negMg_ps[:],
                                    op=ALU.add)
            f_mat = small.tile([NR, B], F32, tag="f_mat")
            nc.scalar.activation(f_mat[:], shift[:], AF.Exp)

            Sf = small.tile([NR, B], F32, tag="Sf")
            nc.vector.tensor_tensor(Sf[:], gath[:, B:2 * B], f_mat[:],
                                    op=ALU.mult)
            St4_ps = ps_m.tile([B, 1], F32, tag="m")
            nc.tensor.matmul(St4_ps[:], Sf[:], ones_col[0:NR, :],
                             start=True, stop=True)
            rS4 = small.tile([B, 1], F32, tag="rS4")
            nc.vector.reciprocal(rS4[:], St4_ps[:])
            rSrow_ps = ps_t.tile([1, B], F32, tag="t")
            nc.tensor.transpose(rSrow_ps[:], rS4[:], ident[0:B, 0:B])
            rSrow = small.tile([1, B], F32, tag="rSrow")
            nc.scalar.copy(rSrow[:], rSrow_ps[:])
            rS8_ps = ps_m.tile([NR, B], F32, tag="m")
            nc.tensor.matmul(rS8_ps[:], ones_row[0:1, 0:NR], rSrow[:],
                             start=True, stop=True)
            f2 = small.tile([NR, B], F32, tag="f2")
            nc.vector.tensor_tensor(f2[:], f_mat[:], rS8_ps[:], op=ALU.mult)
            oc_ps = ps_c.tile([128, B], F32, tag="occ")
            for b in range(B):
                nc.tensor.matmul(oc_ps[:, b:b + 1],
                                 gath[:, 8 + b * D:8 + (b + 1) * D],
                                 f2[:, b:b + 1], start=True, stop=True)

            nc.vector.tensor_copy(results[:, t * B:(t + 1) * B], oc_ps[:])
            qs = work.tile([D, B], F32, tag="qs")
            nc.vector.tensor_tensor(qs[:], qv[:], oc_ps[:], op=ALU.add)
            nc.vector.tensor_scalar(qv[:], qs[:], 0.5, None, op0=ALU.mult)
            if t + 1 < max_steps:
                cf16, cb4 = make_c(qv)

        n_out = max_steps * B
        res_ps = ps_t.tile([n_out, D], F32, tag="t")
        nc.tensor.transpose(res_ps[:], results[:], ident[:])
        res_T = work.tile([n_out, D], F32, tag="resT", bufs=1)
        nc.scalar.copy(res_T[:], res_ps[:])
        nc.sync.dma_start(outs_ap[:], res_T[:])

    nc.compile()
    return nc


def make_inputs(query, documents, Wq, bq, Wk, bk, n_cores: int = N_CORES,
                jpe: int = JPE):
    query = np.asarray(query)
    documents = np.asarray(documents)
    Wq64 = np.asarray(Wq, dtype=np.float64)
    bq64 = np.asarray(bq, dtype=np.float64)
    Wk64 = np.asarray(Wk, dtype=np.float64)

    A2 = Q * (Wk64.T @ Wq64)
    b2 = Q * (Wk64.T @ bq64)
    a2t = np.ascontiguousarray(A2.T.astype(np.float32))
    b2x = np.ascontiguousarray(
        np.repeat(b2.astype(np.float32)[:, None], B, axis=1))
    qv0 = np.ascontiguousarray(
        query.astype(np.float64).mean(axis=1).T.astype(np.float32))
    ident = np.eye(128, dtype=np.float32)

    nl = documents.shape[1] * documents.shape[2]
    dflat = documents.reshape(B, nl, D)
    in_maps = []
    for c in range(n_cores):
        shard = dflat[:, c * MC:(c + 1) * MC, :]
        dt16 = np.ascontiguousarray(
            shard[:, :jpe * 128, :].transpose(0, 2, 1).astype(np.float16))
        # dn4[q, (tile, batch, d)] = docs[b, tile*128+q, d]
        dn4 = np.ascontiguousarray(
            shard.reshape(B, NTILE, 128, D).transpose(2, 1, 0, 3)
            .reshape(128, NTILE * B * D).astype(np.float16))
        in_maps.append({"dt16": dt16, "dn4": dn4, "a2t": a2t, "b2x": b2x,
                        "qv0": qv0, "ident": ident})
    return in_maps


def kernel(query, documents, Wq, bq, Wk, bk, max_steps):
    import time
    from concourse.bass_utils import run_bass_kernel_spmd

    steps = int(max_steps)
    if steps not in _cache:
        _cache[steps] = build(steps)
    nc = _cache[steps]

    in_maps = make_inputs(query, documents, Wq, bq, Wk, bk)
    last_exc = None
    for attempt in range(3):
        try:
            res = run_bass_kernel_spmd(nc, in_maps,
                                       core_ids=list(range(N_CORES)))
            break
        except Exception as e:  # noqa: BLE001
            last_exc = e
            time.sleep(15)
    else:
        raise last_exc
    outs = res.results[0]["outs"]
    return np.ascontiguousarray(
        outs.reshape(steps, B, D).transpose(1, 0, 2))


# revision 3
# speedup vs baseline: 2.0996x; 1.4315x over previous
"""Trainium2 Bass kernel v8 for nn_DynamicReindexingRAG (B=4, N=1024, L=128, D=128, Q=64).

Changes vs v5 (1.58 ms):
- Pool (gpsimd) XYZWC reduces compute per-batch max M_b and sum S_b in one
  instruction each — removes the transpose-heavy DVE max chains.
- Two collectives per step: half A's AllGather is issued mid-step and hides
  under DVE scoring; only half B's is on the critical tail.
- b-major s_dv/w_dv layouts: contiguous ACT exp reads/writes.
- DVE/ACT reduce split with deeper prod rotation (bufs=3) so ACT reduce
  latency doesn't stall DVE multiplies.
"""

import numpy as np

B, N, L, D, Q = 4, 1024, 128, 128, 64
NL = N * L
N_CORES = 8
MC = NL // N_CORES            # m rows per core per batch (16384)
NTILE = MC // 128             # 128 tiles per batch
JPE = 58                      # tiles per batch scored on the PE
NHI = NTILE - JPE             # tiles per batch scored on DVE

_cache = {}


def build(max_steps: int, n_cores: int = N_CORES, jpe: int = JPE, abl=()):
    abl = set(abl)
    import concourse.bass as bass
    import concourse.bacc as bacc
    import concourse.tile as tile
    import concourse.mybir as mybir
    from contextlib import ExitStack

    F32 = mybir.dt.float32
    F16 = mybir.dt.float16
    AF = mybir.ActivationFunctionType
    ALU = mybir.AluOpType
    AX = mybir.AxisListType

    nhi = NTILE - jpe
    pay = 8 + B * D
    GT = 2                        # tiles per DVE work group
    ngrp = nhi // GT

    nc = bacc.Bacc("TRN2", target_bir_lowering=False, debug=False,
                   num_devices=n_cores)
    dt16_ap = nc.dram_tensor("dt16", [B, D, jpe * 128], F16,
                             kind="ExternalInput").ap()
    dn4_ap = nc.dram_tensor("dn4", [128, NTILE * B * D], F16,
                            kind="ExternalInput").ap()
    a2t_ap = nc.dram_tensor("a2t", [D, D], F32, kind="ExternalInput").ap()
    b2x_ap = nc.dram_tensor("b2x", [D, B], F32, kind="ExternalInput").ap()
    qv0_ap = nc.dram_tensor("qv0", [D, B], F32, kind="ExternalInput").ap()
    ident_ap = nc.dram_tensor("ident", [128, 128], F32,
                              kind="ExternalInput").ap()
    outs_ap = nc.dram_tensor("outs", [max_steps * B, D], F32,
                             kind="ExternalOutput").ap()

    with tile.TileContext(nc) as tc, ExitStack() as ctx:
        const = ctx.enter_context(tc.tile_pool(name="const", bufs=1))
        state = ctx.enter_context(tc.tile_pool(name="state", bufs=1))
        work = ctx.enter_context(tc.tile_pool(name="work", bufs=1))
        small = ctx.enter_context(tc.tile_pool(name="small", bufs=2))
        ps_s = ctx.enter_context(tc.tile_pool(name="ps_s", bufs=1, space="PSUM"))
        ps_o = ctx.enter_context(tc.tile_pool(name="ps_o", bufs=1, space="PSUM"))
        ps_t = ctx.enter_context(tc.tile_pool(name="ps_t", bufs=2, space="PSUM"))
        ps_m = ctx.enter_context(tc.tile_pool(name="ps_m", bufs=2, space="PSUM"))
        ps_c = ctx.enter_context(tc.tile_pool(name="ps_c", bufs=1, space="PSUM"))
        dram = ctx.enter_context(tc.tile_pool(name="dram", bufs=1, space="DRAM"))

        # ---- resident docs ----
        dts = []
        for b in range(B):
            dtb = const.tile([128, jpe * 128], F16, tag=f"dtb{b}")
            nc.sync.dma_start(dtb[:], dt16_ap[b])
            dts.append(dtb)
        dn4 = const.tile([128, NTILE * B * D], F16)
        nc.sync.dma_start(dn4[:], dn4_ap[:])

        a2t = const.tile([D, D], F32)
        nc.sync.dma_start(a2t[:], a2t_ap[:])
        b2x = const.tile([D, B], F32)
        nc.sync.dma_start(b2x[:], b2x_ap[:])
        ident = const.tile([128, 128], F32)
        nc.sync.dma_start(ident[:], ident_ap[:])
        ones_row = const.tile([1, 128], F32)
        nc.vector.memset(ones_row[:], 1.0)
        ones_col = const.tile([128, 1], F32)
        nc.vector.memset(ones_col[:], 1.0)

        qv = state.tile([D, B], F32)
        nc.sync.dma_start(qv[:], qv0_ap[:])
        results = state.tile([D, max_steps * B], F32)
        wsum = state.tile([128, 2 * B], F32)

        NR = 2 * n_cores
        cc_in_a = dram.tile([1, pay], F32, tag="cca_i")
        cc_out_a = dram.tile([n_cores, pay], F32, tag="cca_o")
        cc_in_b = dram.tile([1, pay], F32, tag="ccb_i")
        cc_out_b = dram.tile([n_cores, pay], F32, tag="ccb_o")

        def make_c(qv_src):
            # c = A2 @ qv + b2 -> fp16 cols + interleaved row-broadcast
            cq_ps = ps_m.tile([D, B], F32, tag="m")
            nc.tensor.matmul(cq_ps[:], a2t[:], qv_src[:], start=True, stop=True)
            cf32 = work.tile([D, B], F32, tag="cf32")
            nc.vector.tensor_tensor(cf32[:], cq_ps[:], b2x[:], op=ALU.add)
            cf16 = work.tile([D, B], F16, tag="cf16")
            nc.scalar.copy(cf16[:], cf32[:])
            # cb4[q, (b, d)] = c[d, b] broadcast over partitions q
            cb4 = work.tile([128, B * D], F16, tag="cb4", bufs=1)
            for b in range(B):
                ct_ps = ps_t.tile([1, D], F32, tag="t")
                nc.tensor.transpose(ct_ps[:], cf32[:, b:b + 1], ident[:])
                ct_sb = small.tile([1, D], F32, tag="ctsb")
                nc.scalar.copy(ct_sb[:], ct_ps[:])
                cb_ps = ps_t.tile([128, D], F32, tag="t")
                nc.tensor.matmul(cb_ps[:], ones_row[:], ct_sb[:],
                                 start=True, stop=True)
                nc.scalar.copy(cb4[:, b * D:(b + 1) * D], cb_ps[:])
            return cf16, cb4

        cf16, cb4 = make_c(qv)

        for t in range(max_steps):
            # ---- pass 1b: DVE mults; reduces split DVE/ACT; b-major s_dv ----
            s_dv = work.tile([128, B * nhi], F32, tag="sdv", bufs=1)

            def dve_group(g):
                j0 = jpe + g * GT
                src = dn4[:, j0 * B * D:(j0 + GT) * B * D]
                prod = work.tile([128, GT * B * D], F16, tag="prod", bufs=2)
                ch3 = src.rearrange("p (j bd) -> p j bd", j=GT)
                pr3 = prod[:].rearrange("p (j bd) -> p j bd", j=GT)
                cb3 = cb4[:].rearrange("p (o bd) -> p o bd", o=1)
                i0, i1 = bass.broadcast_tensor_aps(ch3, cb3)
                nc.vector.tensor_tensor(pr3, i0, i1, op=ALU.mult)
                pr4 = prod[:].rearrange("p (j b d) -> p j b d", j=GT, b=B)
                if 'dve' in abl:
                    return
                if g % 3 != 2:
                    # reduce on DVE: per batch, segmented over group tiles
                    for b in range(B):
                        nc.vector.tensor_reduce(
                            s_dv[:, b * nhi + g * GT:b * nhi + (g + 1) * GT],
                            pr4[:, :, b, :], axis=AX.X, op=ALU.add)
                else:
                    # reduce on ACT: copy-with-accumulate per tile-batch
                    for jj in range(GT):
                        for b in range(B):
                            scr = work.tile([128, D], F16, tag="scr", bufs=2)
                            nc.scalar.activation(
                                scr[:], prod[:, (jj * B + b) * D:
                                             (jj * B + b + 1) * D], AF.Copy,
                                accum_out=s_dv[:, b * nhi + g * GT + jj:
                                               b * nhi + g * GT + jj + 1])

            if 'dve' in abl:
                nc.vector.memset(s_dv[:], 0.0)
            for g in range(ngrp // 2):
                dve_group(g)

            # ---- pass 1a: PE scores, tiles < jpe; s_pe[q, (b, j)] ----
            s_pe = ps_s.tile([128, B * jpe], F32, tag="s")
            if 'pe1' not in abl:
                for b in range(B):
                    dtb = dts[b]
                    for j in range(jpe):
                        nc.tensor.matmul(
                            s_pe[:, b * jpe + j:b * jpe + j + 1],
                            dtb[:, j * 128:(j + 1) * 128],
                            cf16[:, b:b + 1],
                            start=True, stop=True)
            else:
                nc.tensor.matmul(s_pe[:, 0:1], dts[0][:, 0:128],
                                 cf16[:, 0:1], start=True, stop=True)

            def half_softmax(tag, per_b_views, exp_fn, wsum_cols):
                # per-b full max via Pool XYZWC; neg broadcast; exp; S
                Mrow = small.tile([1, B], F32, tag="Mr" + tag)
                for b in range(B):
                    nc.gpsimd.tensor_reduce(Mrow[0:1, b:b + 1],
                                            per_b_views[b],
                                            axis=AX.XYZWC, op=ALU.max)
                negMrow = small.tile([1, B], F32, tag="nMr" + tag)
                nc.scalar.mul(negMrow[:], Mrow[:], -1.0)
                nM_ps = ps_m.tile([128, B], F32, tag="m")
                nc.tensor.matmul(nM_ps[:], ones_row[:], negMrow[:],
                                 start=True, stop=True)
                negMbc = small.tile([128, B], F32, tag="nMbc" + tag)
                nc.scalar.copy(negMbc[:], nM_ps[:])
                exp_fn(negMbc)
                S4 = small.tile([1, B], F32, tag="S" + tag)
                for b in range(B):
                    nc.gpsimd.tensor_reduce(
                        S4[0:1, b:b + 1],
                        wsum[:, wsum_cols[0] + b:wsum_cols[0] + b + 1],
                        axis=AX.XYZWC, op=ALU.add)
                return Mrow, S4

            def make_payload(tag, Mrow, S4, o4_ps):
                payl = work.tile([1, pay], F32, tag="pay" + tag, bufs=1)
                nc.vector.tensor_copy(payl[0:1, 0:B], Mrow[:])
                nc.vector.tensor_copy(payl[0:1, B:2 * B], S4[:])
                o4_sb = work.tile([B, B * D], F32, tag="o4sb", bufs=1)
                nc.scalar.copy(o4_sb[:], o4_ps[:])
                for b in range(B):
                    orow_ps = ps_m.tile([1, B * D], F32, tag="m")
                    nc.tensor.matmul(orow_ps[:], ident[0:B, b:b + 1],
                                     o4_sb[:], start=True, stop=True)
                    nc.scalar.copy(payl[0:1, 8 + b * D:8 + (b + 1) * D],
                                   orow_ps[0:1, b * D:(b + 1) * D])
                return payl

            # ---- half A: PE-scored tiles ----
            w_pe = work.tile([128, B * jpe], F16, tag="wpe", bufs=1)
            s_pe16 = work.tile([128, B * jpe], F16, tag="spe16", bufs=1)
            nc.scalar.copy(s_pe16[:], s_pe[:])

            def exp_a(negMbc):
                if 'exp' in abl:
                    nc.vector.memset(w_pe[:], 0.0)
                    nc.vector.memset(wsum[:, 0:B], 1.0)
                    return
                for b in range(B):
                    nc.scalar.activation(
                        w_pe[:, b * jpe:(b + 1) * jpe],
                        s_pe[:, b * jpe:(b + 1) * jpe], AF.Exp,
                        bias=negMbc[:, b:b + 1], scale=1.0,
                        accum_out=wsum[:, b:b + 1])

            MrowA, S4A = half_softmax(
                "A", [s_pe16[:, b * jpe:(b + 1) * jpe] for b in range(B)],
                exp_a, (0, B))
            o4a_ps = ps_o.tile([B, B * D], F32, tag="o4a", bufs=1)
            w_pe3 = w_pe[:].rearrange("p (b j) -> p j b", b=B)
            o4a_range = [0] if 'o4' in abl else list(range(jpe))
            for j in o4a_range:
                nc.tensor.matmul(
                    o4a_ps[:], w_pe3[:, j, :],
                    dn4[:, j * B * D:(j + 1) * B * D],
                    start=(j == o4a_range[0]), stop=(j == o4a_range[-1]))
            pay_a = make_payload("a", MrowA, S4A, o4a_ps)
            nc.gpsimd.dma_start(cc_in_a[0:1, :], pay_a[:])
            gath = work.tile([NR, pay], F32, tag="gath", bufs=1)
            if 'cc' not in abl:
                nc.gpsimd.collective_compute(
                    "AllGather", mybir.AluOpType.bypass,
                    replica_groups=[list(range(n_cores))],
                    ins=[cc_in_a.opt()], outs=[cc_out_a.opt()])
            nc.gpsimd.dma_start(gath[0:n_cores, :], cc_out_a[:])

            for g in range(ngrp // 2, ngrp):
                dve_group(g)

            # ---- half B: DVE-scored tiles (b-major layouts) ----
            w_dv = work.tile([128, B * nhi], F16, tag="wdv", bufs=1)

            def exp_b(negMbc):
                if 'exp' in abl:
                    nc.vector.memset(w_dv[:], 0.0)
                    nc.vector.memset(wsum[:, B:2 * B], 1.0)
                    return
                for b in range(B):
                    nc.scalar.activation(
                        w_dv[:, b * nhi:(b + 1) * nhi],
                        s_dv[:, b * nhi:(b + 1) * nhi], AF.Exp,
                        bias=negMbc[:, b:b + 1], scale=1.0,
                        accum_out=wsum[:, B + b:B + b + 1])

            MrowB, S4B = half_softmax(
                "B", [s_dv[:, b * nhi:(b + 1) * nhi] for b in range(B)],
                exp_b, (B, 2 * B))
            o4b_ps = ps_o.tile([B, B * D], F32, tag="o4b", bufs=1)
            w_dv3 = w_dv[:].rearrange("p (b j) -> p j b", b=B)
            o4b_range = [jpe] if 'o4' in abl else list(range(jpe, NTILE))
            for j in o4b_range:
                nc.tensor.matmul(
                    o4b_ps[:], w_dv3[:, j - jpe, :],
                    dn4[:, j * B * D:(j + 1) * B * D],
                    start=(j == o4b_range[0]), stop=(j == o4b_range[-1]))
            pay_b = make_payload("b", MrowB, S4B, o4b_ps)
            nc.gpsimd.dma_start(cc_in_b[0:1, :], pay_b[:])
            if 'cc' not in abl:
                nc.gpsimd.collective_compute(
                    "AllGather", mybir.AluOpType.bypass,
                    replica_groups=[list(range(n_cores))],
                    ins=[cc_in_b.opt()], outs=[cc_out_b.opt()])

            # ---- cross-core combine over NR = 2*n_cores virtual rows ----
            nc.gpsimd.dma_start(gath[n_cores:NR, :], cc_out_b[:])

            Mgrow = small.tile([1, B], F32, tag="Mgrow")
            for b in range(B):
                nc.gpsimd.tensor_reduce(Mgrow[0:1, b:b + 1], gath[:, b:b + 1],
                                        axis=AX.XYZWC, op=ALU.max)
            negMgT = small.tile([1, B], F32, tag="negMgT")
            nc.scalar.mul(negMgT[:], Mgrow[:], -1.0)
            negMg_ps = ps_m.tile([NR, B], F32, tag="m")
            nc.tensor.matmul(negMg_ps[:], ones_row[0:1, 0:NR],
                             negMgT[:], start=True, stop=True)
            shift = small.tile([NR, B], F32, tag="shift")
            nc.vector.tensor_tensor(shift[:], gath[:, 0:B], negMg_ps[:],
                                    op=ALU.add)
            f_mat = small.tile([NR, B], F32, tag="f_mat")
            nc.scalar.activation(f_mat[:], shift[:], AF.Exp)

            Sf = small.tile([NR, B], F32, tag="Sf")
            nc.vector.tensor_tensor(Sf[:], gath[:, B:2 * B], f_mat[:],
                                    op=ALU.mult)
            St4_ps = ps_m.tile([B, 1], F32, tag="m")
            nc.tensor.matmul(St4_ps[:], Sf[:], ones_col[0:NR, :],
                             start=True, stop=True)
            rS4 = small.tile([B, 1], F32, tag="rS4")
            nc.vector.reciprocal(rS4[:], St4_ps[:])
            rSrow_ps = ps_t.tile([1, B], F32, tag="t")
            nc.tensor.transpose(rSrow_ps[:], rS4[:], ident[0:B, 0:B])
            rSrow = small.tile([1, B], F32, tag="rSrow")
            nc.scalar.copy(rSrow[:], rSrow_ps[:])
            rS8_ps = ps_m.tile([NR, B], F32, tag="m")
            nc.tensor.matmul(rS8_ps[:], ones_row[0:1, 0:NR], rSrow[:],
                             start=True, stop=True)
            f2 = small.tile([NR, B], F32, tag="f2")
            nc.vector.tensor_tensor(f2[:], f_mat[:], rS8_ps[:], op=ALU.mult)
            oc_ps = ps_c.tile([128, B], F32, tag="occ")
            for b in range(B):
                nc.tensor.matmul(oc_ps[:, b:b + 1],
                                 gath[:, 8 + b * D:8 + (b + 1) * D],
                                 f2[:, b:b + 1], start=True, stop=True)

            nc.vector.tensor_copy(results[:, t * B:(t + 1) * B], oc_ps[:])
            qs = work.tile([D, B], F32, tag="qs")
            nc.vector.tensor_tensor(qs[:], qv[:], oc_ps[:], op=ALU.add)
            nc.vector.tensor_scalar(qv[:], qs[:], 0.5, None, op0=ALU.mult)
            if t + 1 < max_steps:
                cf16, cb4 = make_c(qv)

        n_out = max_steps * B
        res_ps = ps_t.tile([n_out, D], F32, tag="t")
        nc.tensor.transpose(res_ps[:], results[:], ident[:])
        res_T = work.tile([n_out, D], F32, tag="resT", bufs=1)
        nc.scalar.copy(res_T[:], res_ps[:])
        nc.sync.dma_start(outs_ap[:], res_T[:])

    nc.compile()
    return nc


def make_inputs(query, documents, Wq, bq, Wk, bk, n_cores: int = N_CORES,
                jpe: int = JPE):
    query = np.asarray(query)
    documents = np.asarray(documents)
    Wq64 = np.asarray(Wq, dtype=np.float64)
    bq64 = np.asarray(bq, dtype=np.float64)
    Wk64 = np.asarray(Wk, dtype=np.float64)

    A2 = Q * (Wk64.T @ Wq64)
    b2 = Q * (Wk64.T @ bq64)
    a2t = np.ascontiguousarray(A2.T.astype(np.float32))
    b2x = np.ascontiguousarray(
        np.repeat(b2.astype(np.float32)[:, None], B, axis=1))
    qv0 = np.ascontiguousarray(
        query.astype(np.float64).mean(axis=1).T.astype(np.float32))
    ident = np.eye(128, dtype=np.float32)

    nl = documents.shape[1] * documents.shape[2]
    dflat = documents.reshape(B, nl, D)
    in_maps = []
    for c in range(n_cores):
        shard = dflat[:, c * MC:(c + 1) * MC, :]
        dt16 = np.ascontiguousarray(
            shard[:, :jpe * 128, :].transpose(0, 2, 1).astype(np.float16))
        # dn4[q, (tile, batch, d)] = docs[b, tile*128+q, d]
        dn4 = np.ascontiguousarray(
            shard.reshape(B, NTILE, 128, D).transpose(2, 1, 0, 3)
            .reshape(128, NTILE * B * D).astype(np.float16))
        in_maps.append({"dt16": dt16, "dn4": dn4, "a2t": a2t, "b2x": b2x,
                        "qv0": qv0, "ident": ident})
    return in_maps


def kernel(query, documents, Wq, bq, Wk, bk, max_steps):
    import time
    from concourse.bass_utils import run_bass_kernel_spmd

    steps = int(max_steps)
    if steps not in _cache:
        _cache[steps] = build(steps)
    nc = _cache[steps]

    in_maps = make_inputs(query, documents, Wq, bq, Wk, bk)
    last_exc = None
    for attempt in range(3):
        try:
            res = run_bass_kernel_spmd(nc, in_maps,
                                       core_ids=list(range(N_CORES)))
            break
        except Exception as e:  # noqa: BLE001
            last_exc = e
            time.sleep(15)
    else:
        raise last_exc
    outs = res.results[0]["outs"]
    return np.ascontiguousarray(
        outs.reshape(steps, B, D).transpose(1, 0, 2))
